# revision 1
# baseline (speedup 1.0000x reference)
"""Multi-head self-attention Trainium2 Bass kernel.

Problem: B=4, T=2048, EMB=1024, H=16 heads (head_dim 64), causal mask,
scores scaled by 1/sqrt(EMB), torch-Linear style projections.

Sharding (8 cores): data-parallel over the 4 batches x tensor-parallel over
2 head-groups of 8 heads.  Core c handles batch c//2, heads (c%2)*8..+8.
Each core computes q/k/v projections for its head shard, full TxT causal
attention for its 8 heads, and a partial output projection (its 512 rows of
the unify matmul).  Host sums the two partial outputs per batch and adds bo.

Device layout notes:
 - All PE operands are fp16 (1 cycle/row on the PE); PSUM accumulates fp32.
 - x and the weights are pre-transposed/cast on host so that every matmul
   contraction dim lands on the SBUF partition axis.
 - Scores are computed transposed (pT[s, t] = exp(q.k/32)) so that the
   attn @ v contraction (over s) needs no on-device transposes.  Softmax
   sums ride as a 65th "ones" column of v; normalization happens on the
   small yT tensor via reciprocal + gpsimd partition-broadcast.
"""

import numpy as np

B, T, EMB, H = 4, 2048, 1024, 16
HD = 64          # head dim
HPC = 8          # heads per core
DPC = HPC * HD   # projected dim per core = 512
NCORES = 8
E8 = EMB // 128  # contraction chunks over emb = 8
DP = DPC // 128  # head-pair chunks = 4
TB = T // 512    # t-blocks = 4
SC = T // 128    # s-chunks = 16
VW = HD + 1      # v columns per head incl. ones column = 65
GROUP = 2        # score chunks per exp group (3 PSUM banks)

_CACHED_NC = None
LAST_RESULTS = None  # BassKernelResults of the most recent run (for test.py)


def _build_nc():
    import concourse.bacc as bacc
    import concourse.tile as tile
    import concourse.mybir as mybir

    f16 = mybir.dt.float16
    f32 = mybir.dt.float32
    Exp = mybir.ActivationFunctionType.Exp

    nc = bacc.Bacc(
        "TRN2",
        target_bir_lowering=False,
        debug=False,
        enable_asserts=False,
        num_devices=NCORES,
    )

    xT_d = nc.dram_tensor("xT", [EMB, T], f16, kind="ExternalInput").ap()
    wqT_d = nc.dram_tensor("wqT", [EMB, DPC], f16, kind="ExternalInput").ap()
    wkT_d = nc.dram_tensor("wkT", [EMB, DPC], f16, kind="ExternalInput").ap()
    wvT_d = nc.dram_tensor("wvT", [EMB, DPC], f16, kind="ExternalInput").ap()
    woT_d = nc.dram_tensor("woT", [DPC, EMB], f16, kind="ExternalInput").ap()
    cm_d = nc.dram_tensor("cmask", [128, 2560], f16, kind="ExternalInput").ap()
    out_d = nc.dram_tensor("out", [T, EMB], f32, kind="ExternalOutput").ap()

    with tile.TileContext(nc) as tc:
        # ---- persistent SBUF tensors (static allocations) -------------
        def sb(name, shape):
            return nc.alloc_sbuf_tensor(name, list(shape), f16).ap()

        xt = [sb(f"xt{k}", [128, T]) for k in range(E8)]
        wq = [sb(f"wq{k}", [128, DPC]) for k in range(E8)]
        wk = [sb(f"wk{k}", [128, DPC]) for k in range(E8)]
        wv = [sb(f"wv{k}", [128, DPC]) for k in range(E8)]
        wo = [sb(f"wo{p}", [128, EMB]) for p in range(DP)]
        cm = sb("cm", [128, 2560])
        qt = [sb(f"qt{p}", [128, T]) for p in range(DP)]
        kt = [sb(f"kt{p}", [128, T]) for p in range(DP)]
        vt = sb("vt", [128, SC * HPC * VW])
        ytn = [sb(f"ytn{p}", [128, T]) for p in range(DP)]

        mask_rr = [0]  # round-robin DVE/GpSimd for mask multiplies

        with (
            tc.tile_pool(name="pp", bufs=2, space="PSUM") as pp,
            tc.tile_pool(name="scp", bufs=2, space="PSUM") as scp,
            tc.tile_pool(name="ytp", bufs=1, space="PSUM") as ytp,
            tc.tile_pool(name="ptp", bufs=8) as ptp,
            tc.tile_pool(name="sump", bufs=2) as sump,
            tc.tile_pool(name="recp", bufs=2) as recp,
            tc.tile_pool(name="rrow", bufs=2) as rrow,
            tc.tile_pool(name="brecp", bufs=2) as brecp,
            tc.tile_pool(name="ost", bufs=3) as ost,
        ):
            # ---- input loads (direct DMA, alternating between the two
            # HWDGE queue engines; Bacc legalizes multi-dep matmul waits) -
            load_rr = [0]

            def load(dst, src):
                eng = nc.sync if load_rr[0] % 2 == 0 else nc.scalar
                load_rr[0] += 1
                eng.dma_start(dst, src)

            for k in range(E8):
                r = slice(k * 128, (k + 1) * 128)
                load(xt[k][:, :], xT_d[r, :])
                load(wq[k][:, :], wqT_d[r, :])
            for k in range(E8):
                r = slice(k * 128, (k + 1) * 128)
                load(wk[k][:, :], wkT_d[r, :])
                load(wv[k][:, :], wvT_d[r, :])
            load(cm[:, :], cm_d[:, :])
            for p in range(DP):
                load(wo[p][:, :], woT_d[p * 128:(p + 1) * 128, :])
            # ones columns for the softmax-sum trick
            nc.gpsimd.memset(vt[:, :], 1.0)
            # PE warm-up during the DMA ramp: keeps the HAM activity window
            # busy so real matmuls start at the 2.4 GHz clock.  Results are
            # discarded (scratch PSUM tile, never read).
            warm = pp.tile([128, 512], f32, tag="pp", name="warmup")
            for _ in range(24):
                nc.tensor.matmul(warm[:, :], vt[0:128, 0:128],
                                 vt[0:128, 0:512], start=True, stop=True)

            def project_strip(j):
                """q/k for t-block j (evac on Scalar), v for its s-chunks
                (evac on DVE)."""
                tcols = slice(j * 512, (j + 1) * 512)
                for w_t, dst in ((wq, qt), (wk, kt)):
                    for p in range(DP):
                        dcols = slice(p * 128, (p + 1) * 128)
                        ps = pp.tile([128, 512], f32, tag="pp",
                                     name=f"ps_{p}_{j}")
                        for e in range(E8):
                            nc.tensor.matmul(
                                ps[:, :], w_t[e][:, dcols], xt[e][:, tcols],
                                start=(e == 0), stop=(e == E8 - 1),
                            )
                        nc.scalar.copy(dst[p][:, tcols], ps[:, :])
                for s in range(4 * j, 4 * j + 4):
                    ps = pp.tile([128, 512], f32, tag="pp", name=f"psv_{s}")
                    for e in range(E8):
                        nc.tensor.matmul(
                            ps[:, :], xt[e][:, s * 128:(s + 1) * 128],
                            wv[e][:, :],
                            start=(e == 0), stop=(e == E8 - 1),
                        )
                    dst = vt[:, s * HPC * VW:(s + 1) * HPC * VW]
                    dst = dst.rearrange("p (h c) -> p h c", c=VW)[:, :, 0:HD]
                    src = ps[:, :].rearrange("p (h c) -> p h c", c=HD)
                    nc.vector.tensor_copy(dst, src)

            def attend_block(j):
                tcols = slice(j * 512, (j + 1) * 512)
                nchunks = 4 * j + 4
                # 8 softmax-sum rows at 32-aligned partitions of two tiles
                sums = [
                    sump.tile([97, 512], f32, tag=f"sums{t_}",
                              name=f"sums{t_}_{j}")
                    for t_ in range(2)
                ]
                for t_ in range(2):
                    nc.vector.memset(sums[t_][:, :], 1.0)
                groups = [
                    list(range(g, min(g + GROUP, nchunks)))
                    for g in range(0, nchunks, GROUP)
                ]
                for p in range(DP):
                    yts = [
                        ytp.tile([VW, 512], f32, tag=f"yt{h2}",
                                 name=f"yt{h2}_{p}_{j}")
                        for h2 in range(2)
                    ]
                    for cks in groups:
                        w = 512 * len(cks)
                        scs, pts = [], []
                        for h2 in range(2):
                            scs.append(scp.tile([128, w], f32, tag="sc",
                                       name=f"sc{h2}_{p}_{j}_{cks[0]}"))
                            pts.append(ptp.tile([128, w], f16, tag="pt",
                                       name=f"pt{h2}_{p}_{j}_{cks[0]}"))
                        # scores: alternate the two heads (disjoint PE row
                        # groups) so consecutive matmuls run concurrently
                        for i, ck in enumerate(cks):
                            for h2 in range(2):
                                base = h2 * 64
                                nc.tensor.matmul(
                                    scs[h2][:, i * 512:(i + 1) * 512],
                                    kt[p][base:base + 64,
                                          ck * 128:(ck + 1) * 128],
                                    qt[p][base:base + 64, tcols],
                                    start=True, stop=True,
                                )
                        for h2 in range(2):
                            base = h2 * 64
                            sc, pt = scs[h2], pts[h2]
                            nc.scalar.activation(pt[:, :], sc[:, :], Exp,
                                                 scale=1.0 / 32.0)
                            # Masking, fused across the whole group when
                            # every chunk of the group is diagonal (the
                            # mask blocks are contiguous in cm).
                            dis = [ck - 4 * j for ck in cks]
                            if all(di >= 0 for di in dis):
                                nc.vector.tensor_mul(
                                    pt[:, :], pt[:, :],
                                    cm[:, dis[0] * 512:
                                        (dis[0] + len(cks)) * 512],
                                )
                            else:
                                for i, ck in enumerate(cks):
                                    di = ck - 4 * j
                                    # First accumulated chunk gets an
                                    # all-ones mask (block 4) purely for
                                    # dep shaping.
                                    if di < 0 and ck == 0:
                                        di = 4
                                    if di >= 0:
                                        pcols = slice(i * 512, (i + 1) * 512)
                                        nc.vector.tensor_mul(
                                            pt[:, pcols], pt[:, pcols],
                                            cm[:, di * 512:(di + 1) * 512],
                                        )
                        for i, ck in enumerate(cks):
                            for h2 in range(2):
                                h = 2 * p + h2
                                nc.tensor.matmul(
                                    yts[h2][:, :],
                                    vt[:, ck * HPC * VW + h * VW:
                                       ck * HPC * VW + (h + 1) * VW],
                                    pts[h2][:, i * 512:(i + 1) * 512],
                                    start=(ck == 0), stop=(ck == nchunks - 1),
                                )
                    for h2 in range(2):
                        base = h2 * 64
                        r = p * 2 + h2
                        # quick PSUM evacuation: unnormalized yT + sums row
                        nc.vector.tensor_copy(ytn[p][base:base + 64, tcols],
                                              yts[h2][0:HD, :])
                        row = 32 * (r % 4)
                        nc.vector.tensor_copy(sums[r // 4][row:row + 1, :],
                                              yts[h2][HD:HD + 1, :])
                # deferred normalization for this t-block
                rec = [
                    recp.tile([97, 512], f32, tag=f"rec{t_}",
                              name=f"rec{t_}_{j}")
                    for t_ in range(2)
                ]
                for t_ in range(2):
                    nc.vector.reciprocal(rec[t_][:, :], sums[t_][:, :])
                for p in range(DP):
                    for h2 in range(2):
                        base = h2 * 64
                        r = p * 2 + h2
                        row = 32 * (r % 4)
                        # partition_broadcast needs a base-partition-0 input
                        rr = rrow.tile([1, 512], f32, tag="rr",
                                       name=f"rr{h2}_{p}_{j}")
                        nc.vector.tensor_copy(rr[:, :],
                                              rec[r // 4][row:row + 1, :])
                        brec = brecp.tile([128, 512], f32, tag="brec",
                                          name=f"brec{h2}_{p}_{j}")
                        nc.gpsimd.partition_broadcast(brec[:, :], rr[:, :])
                        nc.vector.tensor_mul(
                            ytn[p][base:base + 64, tcols],
                            ytn[p][base:base + 64, tcols],
                            brec[base:base + 64, :],
                        )

            def outproj_block(j):
                for tcn in range(4 * j, 4 * j + 4):
                    trows = slice(tcn * 128, (tcn + 1) * 128)
                    for n in range(EMB // 512):
                        ncols = slice(n * 512, (n + 1) * 512)
                        ps = pp.tile([128, 512], f32, tag="pp",
                                     name=f"op_{tcn}_{n}")
                        for p in range(DP):
                            nc.tensor.matmul(
                                ps[:, :], ytn[p][:, trows], wo[p][:, ncols],
                                start=(p == 0), stop=(p == DP - 1),
                            )
                        ot = ost.tile([128, 512], f32, tag="ot",
                                      name=f"ot_{tcn}_{n}")
                        nc.vector.tensor_copy(ot[:, :], ps[:, :])
                        nc.sync.dma_start(out_d[trows, ncols], ot[:, :])

            for j in range(TB):
                project_strip(j)
                if j > 0:
                    outproj_block(j - 1)
                attend_block(j)
            outproj_block(TB - 1)

    nc.compile()
    return nc


def _causal_mask_tiles() -> np.ndarray:
    """[128, 2560] fp16: tile i<4 (cols 512i..) is the mask for diagonal
    s-chunk offset i: m[p, c] = 1 if 128*i + p <= c else 0.  Tile 4 is all
    ones (used as a dep-shaping no-op multiply)."""
    m = np.zeros((128, 5, 512), dtype=np.float16)
    p = np.arange(128)[:, None]
    c = np.arange(512)[None, :]
    for i in range(4):
        m[:, i, :] = (128 * i + p <= c).astype(np.float16)
    m[:, 4, :] = 1.0
    return np.ascontiguousarray(m.reshape(128, 2560))


def _numpy_fallback(x, mask, Wq, bq, Wk, bk, Wv, bv, Wo, bo):
    b, t, emb = x.shape
    h = H
    k = emb // h
    q = (x @ Wq.T + bq).reshape(b, t, h, k)
    kk = (x @ Wk.T + bk).reshape(b, t, h, k)
    v = (x @ Wv.T + bv).reshape(b, t, h, k)
    scale = 1.0 / np.sqrt(emb)
    out = np.empty((b, t, emb), dtype=np.float32)
    for bi in range(b):
        yb = np.empty((t, h, k), dtype=np.float32)
        for hi in range(h):
            s = (q[bi, :, hi] @ kk[bi, :, hi].T) * scale
            s = np.where(mask[bi] == 0, np.float32(-1e10), s)
            s = s - s.max(axis=-1, keepdims=True)
            e = np.exp(s)
            p = e / e.sum(axis=-1, keepdims=True)
            yb[:, hi] = p @ v[bi, :, hi]
        out[bi] = yb.reshape(t, emb) @ Wo.T + bo
    return out


def kernel(x, mask, Wq, bq, Wk, bk, Wv, bv, Wo, bo):
    global _CACHED_NC, LAST_RESULTS
    x = np.asarray(x, dtype=np.float32)
    mask = np.asarray(mask)
    Wq, Wk, Wv, Wo = (np.asarray(w, dtype=np.float32) for w in (Wq, Wk, Wv, Wo))
    bq, bk, bv, bo = (np.asarray(v_, dtype=np.float32) for v_ in (bq, bk, bv, bo))

    # The device program hardcodes a causal mask and zero q/k/v biases
    # (which is what reference.setup_inputs produces).  Anything else falls
    # back to a plain numpy implementation.
    tril = np.tril(np.ones((T, T), dtype=mask.dtype))
    if (
        x.shape != (B, T, EMB)
        or any(np.any(bias) for bias in (bq, bk, bv))
        or not all(np.array_equal(np.asarray(mask[b_]), tril) for b_ in range(B))
    ):
        return _numpy_fallback(x, mask, Wq, bq, Wk, bk, Wv, bv, Wo, bo)

    from concourse import bass_utils

    f16 = np.float16
    xT = [np.ascontiguousarray(x[b_].T).astype(f16) for b_ in range(B)]
    cmask = _causal_mask_tiles()
    in_maps = []
    for c in range(NCORES):
        b_, hg = c // 2, c % 2
        r = slice(hg * DPC, (hg + 1) * DPC)
        in_maps.append({
            "xT": xT[b_],
            "wqT": np.ascontiguousarray(Wq[r, :].T).astype(f16),
            "wkT": np.ascontiguousarray(Wk[r, :].T).astype(f16),
            "wvT": np.ascontiguousarray(Wv[r, :].T).astype(f16),
            "woT": np.ascontiguousarray(Wo[:, r].T).astype(f16),
            "cmask": cmask,
        })

    if _CACHED_NC is None:
        _CACHED_NC = _build_nc()

    import os
    trace = bool(int(os.environ.get("KERNEL_TRACE", "0")))
    res = bass_utils.run_bass_kernel_spmd(
        _CACHED_NC,
        in_maps,
        core_ids=list(range(NCORES)),
        trace=trace,
    )
    LAST_RESULTS = res
    outs = [r["out"] for r in res.results]
    y = np.stack([outs[2 * b_] + outs[2 * b_ + 1] for b_ in range(B)])
    y += bo[None, None, :]
    return np.ascontiguousarray(y.astype(np.float32))



# revision 5
# speedup vs baseline: 1.1989x; 1.1989x over previous
"""Multi-head self-attention Trainium2 Bass kernel.

Problem: B=4, T=2048, EMB=1024, H=16 heads (head_dim 64), causal mask,
scores scaled by 1/sqrt(EMB), torch-Linear style projections.

Sharding (8 cores): data-parallel over the 4 batches x tensor-parallel over
2 head-groups of 8 heads.  Core c handles batch c//2, heads (c%2)*8..+8.
Each core computes q/k/v projections for its head shard, full TxT causal
attention for its 8 heads, and a partial output projection (its 512 rows of
the unify matmul).  Host sums the two partial outputs per batch and adds bo.

Device layout notes:
 - All PE operands are fp16 (1 cycle/row on the PE); PSUM accumulates fp32.
 - x and the weights are pre-transposed/cast on host so that every matmul
   contraction dim lands on the SBUF partition axis.
 - Scores are computed transposed (pT[s, t] = exp(q.k/32)) so that the
   attn @ v contraction (over s) needs no on-device transposes.  Softmax
   sums ride as a 65th "ones" column of v; normalization happens on the
   small yT tensor via reciprocal + gpsimd partition-broadcast.
 - Causal structure is exploited at 128-col granularity: for the diagonal
   s-chunk i of a t-block, scores/exp/attn@v only cover t-cols >= 128*i,
   and only the 128-wide boundary strip gets a mask multiply (DVE).
 - Score PSUM tiles hold both heads of a pair ([128, 1024], 2 banks) so a
   single Scalar-engine exp covers both; ~2x fewer ACT instructions.
 - Projection / output-projection tiles are interleaved into the attend
   chunk loop as PE filler so the tensor engine keeps working (and the HAM
   clock stays at 2.4 GHz) while the Scalar engine runs exp.
"""

import numpy as np

B, T, EMB, H = 4, 2048, 1024, 16
HD = 64          # head dim
HPC = 8          # heads per core
DPC = HPC * HD   # projected dim per core = 512
NCORES = 8
E8 = EMB // 128  # contraction chunks over emb = 8
DP = DPC // 128  # head-pair chunks = 4
TB = T // 512    # t-blocks = 4
SC = T // 128    # s-chunks = 16
VW = HD + 1      # v columns per head incl. ones column = 65

_CACHED_NC = None
LAST_RESULTS = None  # BassKernelResults of the most recent run (for test.py)


def _build_nc():
    import concourse.bacc as bacc
    import concourse.tile as tile
    import concourse.mybir as mybir

    f16 = mybir.dt.float16
    f32 = mybir.dt.float32
    Exp = mybir.ActivationFunctionType.Exp

    nc = bacc.Bacc(
        "TRN2",
        target_bir_lowering=False,
        debug=False,
        enable_asserts=False,
        num_devices=NCORES,
    )

    xT_d = nc.dram_tensor("xT", [EMB, T], f16, kind="ExternalInput").ap()
    wqT_d = nc.dram_tensor("wqT", [EMB, DPC], f16, kind="ExternalInput").ap()
    wkT_d = nc.dram_tensor("wkT", [EMB, DPC], f16, kind="ExternalInput").ap()
    wvT_d = nc.dram_tensor("wvT", [EMB, DPC], f16, kind="ExternalInput").ap()
    woT_d = nc.dram_tensor("woT", [DPC, EMB], f16, kind="ExternalInput").ap()
    cmb_d = nc.dram_tensor("cmb", [128, 128], f16, kind="ExternalInput").ap()
    out_d = nc.dram_tensor("out", [T, EMB], f32, kind="ExternalOutput").ap()

    with tile.TileContext(nc) as tc:
        # ---- persistent SBUF tensors (static allocations) -------------
        def sb(name, shape):
            return nc.alloc_sbuf_tensor(name, list(shape), f16).ap()

        xt = [sb(f"xt{k}", [128, T]) for k in range(E8)]
        wq = [sb(f"wq{k}", [128, DPC]) for k in range(E8)]
        wk = [sb(f"wk{k}", [128, DPC]) for k in range(E8)]
        wv = [sb(f"wv{k}", [128, DPC]) for k in range(E8)]
        wo = [sb(f"wo{p}", [128, EMB]) for p in range(DP)]
        cmb = sb("cmb_sb", [128, 128])
        qt = [sb(f"qt{p}", [128, T]) for p in range(DP)]
        kt = [sb(f"kt{p}", [128, T]) for p in range(DP)]
        vt = sb("vt", [128, SC * HPC * VW])
        ytn = [sb(f"ytn{p}", [128, T]) for p in range(DP)]

        with (
            tc.tile_pool(name="pp", bufs=2, space="PSUM") as pp,
            tc.tile_pool(name="scp", bufs=2, space="PSUM") as scp,
            tc.tile_pool(name="ytp", bufs=1, space="PSUM") as ytp,
            tc.tile_pool(name="ptp", bufs=6) as ptp,
            tc.tile_pool(name="sump", bufs=2) as sump,
            tc.tile_pool(name="recp", bufs=2) as recp,
            tc.tile_pool(name="rrow", bufs=2) as rrow,
            tc.tile_pool(name="brecp", bufs=2) as brecp,
            tc.tile_pool(name="ost", bufs=3) as ost,
        ):
            # ---- input loads (direct DMA, alternating between the two
            # HWDGE queue engines) ----------------------------------------
            load_rr = [0]

            def load(dst, src):
                eng = nc.sync if load_rr[0] % 2 == 0 else nc.scalar
                load_rr[0] += 1
                eng.dma_start(dst, src)

            # weights + x for t-block 0 first so proj(0) can start early;
            # remaining x column-slices stream in behind.
            for k in range(E8):
                r = slice(k * 128, (k + 1) * 128)
                load(wq[k][:, :], wqT_d[r, :])
                load(xt[k][:, 0:512], xT_d[r, 0:512])
            for k in range(E8):
                r = slice(k * 128, (k + 1) * 128)
                load(wk[k][:, :], wkT_d[r, :])
                load(wv[k][:, :], wvT_d[r, :])
            load(cmb[:, :], cmb_d[:, :])
            for p in range(DP):
                load(wo[p][:, :], woT_d[p * 128:(p + 1) * 128, :])
            for j in range(1, TB):
                tcols = slice(j * 512, (j + 1) * 512)
                for k in range(E8):
                    r = slice(k * 128, (k + 1) * 128)
                    load(xt[k][:, tcols], xT_d[r, tcols])

            # ones columns for the softmax-sum trick
            nc.gpsimd.memset(vt[:, :], 1.0)
            # PE warm-up during the DMA ramp: keeps the HAM activity window
            # busy so real matmuls start at the 2.4 GHz clock.
            warm = pp.tile([128, 512], f32, tag="pp", name="warmup")
            for _ in range(24):
                nc.tensor.matmul(warm[:, :], vt[0:128, 0:128],
                                 vt[0:128, 0:512], start=True, stop=True)

            # ---- PE tile tasks (closures, so attend can interleave) -----
            def qk_task(w_t, dst, p, j, lbl):
                def run():
                    tcols = slice(j * 512, (j + 1) * 512)
                    dcols = slice(p * 128, (p + 1) * 128)
                    ps = pp.tile([128, 512], f32, tag="pp",
                                 name=f"ps_{lbl}_{p}_{j}")
                    for e in range(E8):
                        nc.tensor.matmul(
                            ps[:, :], w_t[e][:, dcols], xt[e][:, tcols],
                            start=(e == 0), stop=(e == E8 - 1),
                        )
                    nc.vector.tensor_copy(dst[p][:, tcols], ps[:, :])
                return run

            def v_task(s, j):
                def run():
                    ps = pp.tile([128, 512], f32, tag="pp", name=f"psv_{s}")
                    for e in range(E8):
                        nc.tensor.matmul(
                            ps[:, :], xt[e][:, s * 128:(s + 1) * 128],
                            wv[e][:, :],
                            start=(e == 0), stop=(e == E8 - 1),
                        )
                    dst = vt[:, s * HPC * VW:(s + 1) * HPC * VW]
                    dst = dst.rearrange("p (h c) -> p h c", c=VW)[:, :, 0:HD]
                    src = ps[:, :].rearrange("p (h c) -> p h c", c=HD)
                    nc.vector.tensor_copy(dst, src)
                return run

            def proj_tasks(j):
                tasks = []
                for w_t, dst, lbl in ((wq, qt, "q"), (wk, kt, "k")):
                    for p in range(DP):
                        tasks.append(qk_task(w_t, dst, p, j, lbl))
                for s in range(4 * j, 4 * j + 4):
                    tasks.append(v_task(s, j))
                return tasks

            def op_task(tcn, n):
                def run():
                    trows = slice(tcn * 128, (tcn + 1) * 128)
                    ncols = slice(n * 512, (n + 1) * 512)
                    ps = pp.tile([128, 512], f32, tag="pp",
                                 name=f"op_{tcn}_{n}")
                    for p in range(DP):
                        nc.tensor.matmul(
                            ps[:, :], ytn[p][:, trows], wo[p][:, ncols],
                            start=(p == 0), stop=(p == DP - 1),
                        )
                    ot = ost.tile([128, 512], f32, tag="ot",
                                  name=f"ot_{tcn}_{n}")
                    nc.vector.tensor_copy(ot[:, :], ps[:, :])
                    nc.sync.dma_start(out_d[trows, ncols], ot[:, :])
                return run

            def outproj_tasks(j):
                return [op_task(tcn, n)
                        for tcn in range(4 * j, 4 * j + 4)
                        for n in range(EMB // 512)]

            def attend_block(j, fillers):
                tcols = slice(j * 512, (j + 1) * 512)
                nchunks = 4 * j + 4
                total = DP * nchunks
                done = [0, 0]  # chunks emitted, fillers popped

                def pace():
                    done[0] += 1
                    want = len(fillers) * done[0] // total
                    while done[1] < want:
                        fillers[done[1]]()
                        done[1] += 1

                # 8 softmax-sum rows at 32-aligned partitions of two tiles
                sums = [
                    sump.tile([97, 512], f32, tag=f"sums{t_}",
                              name=f"sums{t_}_{j}")
                    for t_ in range(2)
                ]
                for t_ in range(2):
                    nc.vector.memset(sums[t_][:, :], 1.0)
                for p in range(DP):
                    yts = [
                        ytp.tile([VW, 512], f32, tag=f"yt{h2}",
                                 name=f"yt{h2}_{p}_{j}")
                        for h2 in range(2)
                    ]
                    for ck in range(nchunks):
                        i = ck - 4 * j           # >=0 -> diagonal chunk
                        trim = 128 * i if i > 0 else 0
                        w = 512 - trim           # active t-cols
                        scs = scp.tile([128, 1024], f32, tag="sc",
                                       name=f"sc_{p}_{j}_{ck}")
                        for h2 in range(2):
                            base = h2 * 64
                            off = h2 * 512
                            nc.tensor.matmul(
                                scs[:, off + trim:off + 512],
                                kt[p][base:base + 64,
                                      ck * 128:(ck + 1) * 128],
                                qt[p][base:base + 64,
                                      j * 512 + trim:(j + 1) * 512],
                                start=True, stop=True,
                            )
                        # one exp for both heads ([128, 2, w] strided AP)
                        pt = ptp.tile([128, 1024], f16, tag="pt",
                                      name=f"pt_{p}_{j}_{ck}")
                        src = scs[:, :].rearrange(
                            "p (h w) -> p h w", w=512)[:, :, trim:]
                        dst = pt[:, :].rearrange(
                            "p (h w) -> p h w", w=512)[:, :, trim:]
                        nc.scalar.activation(dst, src, Exp, scale=1.0 / 32.0)
                        if i >= 0:
                            # boundary strip mask (cols trim..trim+128)
                            for h2 in range(2):
                                pcols = slice(h2 * 512 + trim,
                                              h2 * 512 + trim + 128)
                                nc.vector.tensor_mul(
                                    pt[:, pcols], pt[:, pcols], cmb[:, :])
                        for h2 in range(2):
                            h = 2 * p + h2
                            nc.tensor.matmul(
                                yts[h2][:, trim:512],
                                vt[:, ck * HPC * VW + h * VW:
                                   ck * HPC * VW + (h + 1) * VW],
                                pt[:, h2 * 512 + trim:(h2 + 1) * 512],
                                start=(ck == 0), stop=(ck == nchunks - 1),
                            )
                        pace()
                    for h2 in range(2):
                        base = h2 * 64
                        r = p * 2 + h2
                        # quick PSUM evacuation: unnormalized yT + sums row
                        nc.vector.tensor_copy(ytn[p][base:base + 64, tcols],
                                              yts[h2][0:HD, :])
                        row = 32 * (r % 4)
                        nc.vector.tensor_copy(sums[r // 4][row:row + 1, :],
                                              yts[h2][HD:HD + 1, :])
                # deferred normalization for this t-block
                rec = [
                    recp.tile([97, 512], f32, tag=f"rec{t_}",
                              name=f"rec{t_}_{j}")
                    for t_ in range(2)
                ]
                for t_ in range(2):
                    nc.vector.reciprocal(rec[t_][:, :], sums[t_][:, :])
                for p in range(DP):
                    for h2 in range(2):
                        base = h2 * 64
                        r = p * 2 + h2
                        row = 32 * (r % 4)
                        # partition_broadcast needs a base-partition-0 input
                        rr = rrow.tile([1, 512], f32, tag="rr",
                                       name=f"rr{h2}_{p}_{j}")
                        nc.vector.tensor_copy(rr[:, :],
                                              rec[r // 4][row:row + 1, :])
                        brec = brecp.tile([128, 512], f32, tag="brec",
                                          name=f"brec{h2}_{p}_{j}")
                        nc.gpsimd.partition_broadcast(brec[:, :], rr[:, :])
                        nc.vector.tensor_mul(
                            ytn[p][base:base + 64, tcols],
                            ytn[p][base:base + 64, tcols],
                            brec[base:base + 64, :],
                        )
                return done[1]

            for t in proj_tasks(0):
                t()
            for j in range(TB):
                fillers = []
                if j + 1 < TB:
                    fillers += proj_tasks(j + 1)
                if j > 0:
                    fillers += outproj_tasks(j - 1)
                popped = attend_block(j, fillers)
                for t in fillers[popped:]:
                    t()
            for t in outproj_tasks(TB - 1):
                t()

    nc.compile()
    return nc


def _boundary_mask() -> np.ndarray:
    """[128, 128] fp16 inclusive-diagonal mask for the diagonal boundary
    strip of a diagonal s-chunk: m[s, c] = 1 if s <= c else 0."""
    p = np.arange(128)[:, None]
    c = np.arange(128)[None, :]
    return np.ascontiguousarray((p <= c).astype(np.float16))


def _numpy_fallback(x, mask, Wq, bq, Wk, bk, Wv, bv, Wo, bo):
    b, t, emb = x.shape
    h = H
    k = emb // h
    q = (x @ Wq.T + bq).reshape(b, t, h, k)
    kk = (x @ Wk.T + bk).reshape(b, t, h, k)
    v = (x @ Wv.T + bv).reshape(b, t, h, k)
    out = np.empty((b, t, emb), dtype=np.float32)
    for bi in range(b):
        yb = np.empty((t, h, k), dtype=np.float32)
        for hi in range(h):
            s = (q[bi, :, hi] @ kk[bi, :, hi].T) / np.sqrt(emb)
            s = np.where(mask[bi] == 0, np.float32(-1e10), s)
            s = s - s.max(axis=-1, keepdims=True)
            e = np.exp(s)
            p = e / e.sum(axis=-1, keepdims=True)
            yb[:, hi] = p @ v[bi, :, hi]
        out[bi] = yb.reshape(t, emb) @ Wo.T + bo
    return out


def kernel(x, mask, Wq, bq, Wk, bk, Wv, bv, Wo, bo):
    global _CACHED_NC, LAST_RESULTS
    x = np.asarray(x, dtype=np.float32)
    mask = np.asarray(mask)
    Wq, Wk, Wv, Wo = (np.asarray(w, dtype=np.float32) for w in (Wq, Wk, Wv, Wo))
    bq, bk, bv, bo = (np.asarray(v_, dtype=np.float32) for v_ in (bq, bk, bv, bo))

    # The device program hardcodes a causal mask and zero q/k/v biases
    # (which is what reference.setup_inputs produces).  Anything else falls
    # back to a plain numpy implementation.
    tril = np.tril(np.ones((T, T), dtype=mask.dtype))
    if (
        x.shape != (B, T, EMB)
        or any(np.any(bias) for bias in (bq, bk, bv))
        or not all(np.array_equal(np.asarray(mask[b_]), tril) for b_ in range(B))
    ):
        return _numpy_fallback(x, mask, Wq, bq, Wk, bk, Wv, bv, Wo, bo)

    from concourse import bass_utils

    f16 = np.float16
    xT = [np.ascontiguousarray(x[b_].T).astype(f16) for b_ in range(B)]
    cmb = _boundary_mask()
    in_maps = []
    for c in range(NCORES):
        b_, hg = c // 2, c % 2
        r = slice(hg * DPC, (hg + 1) * DPC)
        in_maps.append({
            "xT": xT[b_],
            "wqT": np.ascontiguousarray(Wq[r, :].T).astype(f16),
            "wkT": np.ascontiguousarray(Wk[r, :].T).astype(f16),
            "wvT": np.ascontiguousarray(Wv[r, :].T).astype(f16),
            "woT": np.ascontiguousarray(Wo[:, r].T).astype(f16),
            "cmb": cmb,
        })

    if _CACHED_NC is None:
        _CACHED_NC = _build_nc()

    import os
    trace = bool(int(os.environ.get("KERNEL_TRACE", "0")))
    res = bass_utils.run_bass_kernel_spmd(
        _CACHED_NC,
        in_maps,
        core_ids=list(range(NCORES)),
        trace=trace,
    )
    LAST_RESULTS = res
    outs = [r["out"] for r in res.results]
    y = np.stack([outs[2 * b_] + outs[2 * b_ + 1] for b_ in range(B)])
    y += bo[None, None, :]
    return np.ascontiguousarray(y.astype(np.float32))
